# revision 17
# baseline (speedup 1.0000x reference)
"""Child-Sum TreeLSTM over complete binary trees — Trainium2 Bass kernel.

Sharding: data-parallel over the batch-of-trees axis B=32 across 8 NeuronCores
(4 trees/core); the 8 gate weight matrices are replicated.

Per-core dataflow (activations kept feature-transposed in SBUF as
[feat-chunk(128,128,44), 3, cols] tiles; weights natural = lhsT):
  - level-by-level bottom-up; per <=512-column block:
      embs^T loaded by XBAR DMA-transpose directly from a host-prepared
      bf16 padded copy of embs ([..., 384] with feature 300 = 1.0 ones row)
      for levels >= 7; PE-transpose path for the small deep levels
      gate pre-acts accumulate in PSUM over 6 K-chunks: x-side bf16 +
      h-side float32r; the combined bias (bx+bh) rides as a 45th weight row
      against the baked-in ones row
      sigma/tanh evacuate PSUM->SBUF in one ACT instruction per gate
      per-child forget gates use a step-0 duplicated rhs (each parent column
      streamed twice) so fx lands directly at child granularity
      c_new = i*u + f1*c1 + f2*c2 and h = o*tanh(c) on DVE
      h^T -> PE-transpose -> natural -> DMA to output
  - levels 10/9/8 spill h^T/c^T through internal DRAM (SBUF pressure);
    levels <= 7 keep h^T/c^T resident in SBUF
  - matmuls with moving dim < 256 (deep levels) switch the h-side to bf16
    (float32r drops to 4 cycles/row below 256).
"""

import numpy as np
import ml_dtypes

import concourse.bass as bass
import concourse.mybir as mybir
import concourse.tile as tile
from concourse import bacc
from concourse.masks import make_identity
from concourse.bass_utils import run_bass_kernel_spmd

F32 = mybir.dt.float32
F32R = mybir.dt.float32r
BF16 = mybir.dt.bfloat16
AF = mybir.ActivationFunctionType

B, D, DIM = 32, 11, 300
N = 2**D - 1          # 2047
CORES = 8
BL = B // CORES       # trees per core
KS = [128, 128, 44]   # feature chunks of 300
KO = [0, 128, 256]
NBMAX = 512
SPILL_LV = (10, 9, 8)
SPOFF = {10: 0, 9: BL * 1024, 8: BL * 1024 + BL * 512}
SPTOT = BL * 1024 + BL * 512 + BL * 256
PROJ = {"i": 0, "f": 1, "o": 2, "u": 3}

_NC_CACHE = []


def _cols(l):
    return BL * (1 << l)


def _build():
    nc = bacc.Bacc("TRN2", target_bir_lowering=False, debug=False,
                   num_devices=CORES)
    embs = nc.dram_tensor("embs", [BL, N, DIM], F32, kind="ExternalInput")
    WX = nc.dram_tensor("wx", [128, 4, 3, DIM], F32R, kind="ExternalInput")
    WH = nc.dram_tensor("wh", [128, 4, 3, DIM], F32R, kind="ExternalInput")
    hout = nc.dram_tensor("hout", [BL, N, DIM], F32, kind="ExternalOutput")
    sph = nc.dram_tensor("sph", [128, 3, SPTOT], F32R)
    spc = nc.dram_tensor("spc", [128, 3, SPTOT], F32)

    with tile.TileContext(nc) as tc:
        import contextlib
        with contextlib.ExitStack() as ctx:
            sb = ctx.enter_context(tc.tile_pool(name="sb", bufs=1))
            exp = ctx.enter_context(tc.tile_pool(name="exp", bufs=2))
            xtp = ctx.enter_context(tc.tile_pool(name="xtp", bufs=2))
            hsp = ctx.enter_context(tc.tile_pool(name="hsp", bufs=2))
            gp = ctx.enter_context(tc.tile_pool(name="gp", bufs=6))
            fcp = ctx.enter_context(tc.tile_pool(name="fcp", bufs=2))
            onp = ctx.enter_context(tc.tile_pool(name="onp", bufs=3))
            hcb = ctx.enter_context(tc.tile_pool(name="hcb", bufs=4))
            rbp = ctx.enter_context(tc.tile_pool(name="rbp", bufs=2))
            stp = ctx.enter_context(tc.tile_pool(name="stp", bufs=1))
            psum = ctx.enter_context(
                tc.tile_pool(name="psum", bufs=2, space="PSUM"))

            ident = sb.tile([128, 128], F32)
            make_identity(nc, ident[:, :])

            wx_t = sb.tile([128, 4, 3, DIM], F32R, name="wx_t")
            wh_t = sb.tile([128, 4, 3, DIM], F32R, name="wh_t")
            nc.sync.dma_start(out=wx_t[:, :, :, :], in_=WX[:, :, :, :])
            nc.sync.dma_start(out=wh_t[:, :, :, :], in_=WH[:, :, :, :])
            wx = {nm: wx_t[:, p] for nm, p in PROJ.items()}
            wh = {nm: wh_t[:, p] for nm, p in PROJ.items()}

            def nat_ap(dram, l, r0, rs, w):
                base = (1 << l) - 1
                if l >= 7:
                    t, j0 = r0 >> l, r0 & ((1 << l) - 1)
                    return dram[t, base + j0: base + j0 + rs, 0:w]
                t0, tcnt = r0 >> l, rs >> l
                return dram[t0:t0 + tcnt, base:base + (1 << l), 0:w]

            def load_ex(l, c0, nb):
                """embs^T (+ones row) for parent cols [c0, c0+nb), bf16."""
                ex = exp.tile([128, 3, NBMAX], F32R, tag="ex")
                pT = psum.tile([128, 3, NBMAX], F32, tag="big")
                for r0 in range(0, nb, 128):
                    rs = min(128, nb - r0)
                    xt = xtp.tile([128, 304], F32, tag="xt")
                    nc.gpsimd.memset(xt[:, 300:304], 1.0)
                    nc.sync.dma_start(out=xt[0:rs, 0:300],
                                      in_=nat_ap(embs, l, c0 + r0, rs, DIM))
                    for f in range(3):
                        ke = KS[f] + (1 if f == 2 else 0)
                        nc.tensor.transpose(
                            out=pT[0:ke, f, r0:r0 + rs],
                            in_=xt[0:rs, KO[f]:KO[f] + ke],
                            identity=ident[0:rs, 0:rs])
                nc.scalar.copy(ex[0:128, 0, 0:nb], pT[0:128, 0, 0:nb])
                nc.scalar.copy(ex[0:128, 1, 0:nb], pT[0:128, 1, 0:nb])
                nc.scalar.copy(ex[0:45, 2, 0:nb], pT[0:45, 2, 0:nb])
                return ex

            def store_nat(l, c0, nb, hsrc, s0):
                for r0 in range(0, nb, 128):
                    rs = min(128, nb - r0)
                    pO = psum.tile([128, 304], F32, tag="oT")
                    for f in range(3):
                        nc.tensor.transpose(
                            out=pO[0:rs, KO[f]:KO[f] + KS[f]],
                            in_=hsrc[0:KS[f], f,
                                     s0 + r0:s0 + r0 + rs].bitcast(F32),
                            identity=ident[0:KS[f], 0:KS[f]])
                    on = onp.tile([128, 300], F32, tag="on")
                    nc.scalar.copy(on[0:rs, :], pO[0:rs, 0:300])
                    nc.gpsimd.dma_start(out=nat_ap(hout, l, c0 + r0, rs,
                                                   DIM),
                                        in_=on[0:rs, :])

            st_h = {l: stp.tile([128, 3, _cols(l)], F32R, tag=f"sh{l}",
                                name=f"sh{l}") for l in range(0, 8)}
            st_c = {l: stp.tile([128, 3, _cols(l)], F32, tag=f"sc{l}",
                                name=f"sc{l}") for l in range(0, 8)}

            # ---------------- leaves (level 10) ----------------
            l = 10
            for c0 in range(0, _cols(l), NBMAX):
                nb = NBMAX
                ex = load_ex(l, c0, nb)
                sg = {}
                for nm, fn in (("i", AF.Sigmoid), ("o", AF.Sigmoid),
                               ("u", AF.Tanh)):
                    pG = psum.tile([128, 3, NBMAX], F32, tag="big")
                    for m in range(3):
                        ms, mo = KS[m], KO[m]
                        for k in range(3):
                            kx = KS[k] + (1 if k == 2 else 0)
                            nc.tensor.matmul(
                                pG[0:ms, m, 0:nb],
                                wx[nm][0:kx, k, mo:mo + ms],
                                ex[0:kx, k, 0:nb],
                                start=(k == 0), stop=(k == 2))
                    g = gp.tile([128, 3, NBMAX], F32, tag="g")
                    nc.scalar.activation(g[:, :, 0:nb], pG[:, :, 0:nb], fn)
                    sg[nm] = g
                cb = hcb.tile([128, 3, NBMAX], F32, tag="cb")
                hb = hcb.tile([128, 3, NBMAX], F32R, tag="hb")
                nc.vector.tensor_mul(cb[:, :, 0:nb], sg["i"][:, :, 0:nb],
                                     sg["u"][:, :, 0:nb])
                th = gp.tile([128, 3, NBMAX], F32, tag="g")
                nc.scalar.activation(th[:, :, 0:nb], cb[:, :, 0:nb], AF.Tanh)
                nc.vector.tensor_mul(hb[:, :, 0:nb], sg["o"][:, :, 0:nb],
                                     th[:, :, 0:nb])
                off = SPOFF[l] + c0
                nc.gpsimd.dma_start(out=sph[:, :, off:off + nb],
                                    in_=hb[:, :, 0:nb])
                nc.gpsimd.dma_start(out=spc[:, :, off:off + nb],
                                    in_=cb[:, :, 0:nb])
                store_nat(l, c0, nb, hb, 0)

            # ---------------- internal levels 9..0 ----------------
            for l in range(9, -1, -1):
                cols = _cols(l)
                spill = l in SPILL_LV
                child_spill = (l + 1) in SPILL_LV
                for c0 in range(0, cols, NBMAX):
                    nb = min(NBMAX, cols - c0)
                    fs = min(2 * nb, NBMAX)
                    nsub = (2 * nb) // fs
                    ex = load_ex(l, c0, nb)

                    hn, cn = [], []
                    for s in range(nsub):
                        ch0 = 2 * c0 + s * fs
                        if child_spill:
                            t_h = rbp.tile([128, 3, NBMAX], F32R, tag="rh")
                            t_c = rbp.tile([128, 3, NBMAX], F32, tag="rc")
                            off = SPOFF[l + 1] + ch0
                            nc.sync.dma_start(out=t_h[:, :, 0:fs],
                                              in_=sph[:, :, off:off + fs])
                            nc.sync.dma_start(out=t_c[:, :, 0:fs],
                                              in_=spc[:, :, off:off + fs])
                            hn.append((t_h, 0))
                            cn.append((t_c, 0))
                        else:
                            hn.append((st_h[l + 1], ch0))
                            cn.append((st_c[l + 1], ch0))

                    hs = hsp.tile([128, 3, NBMAX], F32R, tag="hs",
                                  name=f"hs_{l}_{c0}")
                    for s in range(nsub):
                        t_h, o_h = hn[s]
                        pair = t_h[:, :, o_h:o_h + fs].rearrange(
                            "p c (n two) -> p c n two", two=2)
                        nc.vector.tensor_add(
                            hs[:, :, s * fs // 2:(s + 1) * fs // 2],
                            pair[:, :, :, 0], pair[:, :, :, 1])

                    sg = {}
                    for nm, fn in (("i", AF.Sigmoid), ("o", AF.Sigmoid),
                                   ("u", AF.Tanh)):
                        pG = psum.tile([128, 3, NBMAX], F32, tag="big")
                        for m in range(3):
                            ms, mo = KS[m], KO[m]
                            for k in range(3):
                                kx = KS[k] + (1 if k == 2 else 0)
                                nc.tensor.matmul(
                                    pG[0:ms, m, 0:nb],
                                    wx[nm][0:kx, k, mo:mo + ms],
                                    ex[0:kx, k, 0:nb],
                                    start=(k == 0), stop=False)
                            for k in range(3):
                                nc.tensor.matmul(
                                    pG[0:ms, m, 0:nb],
                                    wh[nm][0:KS[k], k, mo:mo + ms],
                                    hs[0:KS[k], k, 0:nb],
                                    start=False, stop=(k == 2))
                        g = gp.tile([128, 3, NBMAX], F32, tag="g")
                        nc.scalar.activation(g[:, :, 0:nb], pG[:, :, 0:nb], fn)
                        sg[nm] = g

                    if spill:
                        cdst = hcb.tile([128, 3, NBMAX], F32, tag="cb")
                        hdst = hcb.tile([128, 3, NBMAX], F32R, tag="hb")
                        d0 = 0
                    else:
                        cdst, hdst, d0 = st_c[l], st_h[l], c0

                    cc = cdst[:, :, d0:d0 + nb]
                    nc.vector.tensor_mul(cc, sg["i"][:, :, 0:nb],
                                         sg["u"][:, :, 0:nb])

                    for s in range(nsub):
                        pF = psum.tile([128, 3, NBMAX], F32, tag="big")
                        p0 = s * fs // 2
                        w_h = wh["f"]
                        t_h, o_h = hn[s]
                        for m in range(3):
                            ms, mo = KS[m], KO[m]
                            for k in range(3):
                                kx = KS[k] + (1 if k == 2 else 0)
                                dup = ex[0:kx, k, p0:p0 + fs // 2] \
                                    .unsqueeze(2).broadcast_to([kx, fs // 2, 2])
                                nc.tensor.matmul(
                                    pF[0:ms, m, 0:fs],
                                    wx["f"][0:kx, k, mo:mo + ms], dup,
                                    start=(k == 0), stop=False)
                            for k in range(3):
                                nc.tensor.matmul(
                                    pF[0:ms, m, 0:fs],
                                    w_h[0:KS[k], k, mo:mo + ms],
                                    t_h[0:KS[k], k, o_h:o_h + fs],
                                    start=False, stop=(k == 2))
                        fg = gp.tile([128, 3, NBMAX], F32, tag="g")
                        nc.scalar.activation(fg[:, :, 0:fs], pF[:, :, 0:fs],
                                             AF.Sigmoid)
                        t_c, o_c = cn[s]
                        fc = fcp.tile([128, 3, NBMAX], F32, tag="fc")
                        nc.vector.tensor_mul(fc[:, :, 0:fs],
                                             fg[:, :, 0:fs],
                                             t_c[:, :, o_c:o_c + fs])
                        pair = fc[:, :, 0:fs].rearrange(
                            "p c (n two) -> p c n two", two=2)
                        ccs = cdst[:, :, d0 + p0:d0 + p0 + fs // 2]
                        nc.vector.tensor_add(ccs, ccs, pair[:, :, :, 0])
                        nc.vector.tensor_add(ccs, ccs, pair[:, :, :, 1])

                    th = gp.tile([128, 3, NBMAX], F32, tag="g")
                    nc.scalar.activation(th[:, :, 0:nb], cc, AF.Tanh)
                    nc.vector.tensor_mul(hdst[:, :, d0:d0 + nb],
                                         sg["o"][:, :, 0:nb], th[:, :, 0:nb])

                    if spill:
                        off = SPOFF[l] + c0
                        nc.gpsimd.dma_start(out=sph[:, :, off:off + nb],
                                            in_=hdst[:, :, 0:nb])
                        nc.gpsimd.dma_start(out=spc[:, :, off:off + nb],
                                            in_=cdst[:, :, 0:nb])
                    store_nat(l, c0, nb, hdst, d0)
    nc.compile()
    return nc


def kernel(embs, Wix, bix, Wih, bih, Wfx, bfx, Wfh, bfh,
           Wox, box, Woh, boh, Wux, bux, Wuh, buh):
    embs = np.ascontiguousarray(np.asarray(embs, dtype=np.float32))
    if not _NC_CACHE:
        _NC_CACHE.append(_build())
    nc = _NC_CACHE[0]

    def chunked(stack, bias_rows):
        out = np.zeros((128, 4, 3, DIM), np.float32)
        for p in range(4):
            out[0:128, p, 0] = stack[p][0:128]
            out[0:128, p, 1] = stack[p][128:256]
            out[0:44, p, 2] = stack[p][256:300]
            if bias_rows is not None:
                out[44, p, 2] = bias_rows[p]
        return out

    xw = [np.asarray(w, np.float32) for w in (Wix, Wfx, Wox, Wux)]
    xb = [np.asarray(bix) + np.asarray(bih), np.asarray(bfx) + np.asarray(bfh),
          np.asarray(box) + np.asarray(boh), np.asarray(bux) + np.asarray(buh)]
    hw_ = [np.asarray(w, np.float32) for w in (Wih, Wfh, Woh, Wuh)]
    wxp = chunked(xw, xb)
    whp = chunked(hw_, None)

    in_maps = [{"embs": embs[c * BL:(c + 1) * BL],
                "wx": wxp, "wh": whp}
               for c in range(CORES)]
    res = run_bass_kernel_spmd(nc, in_maps, list(range(CORES)))
    return np.concatenate([res.results[c]["hout"] for c in range(CORES)],
                          axis=0)


# revision 18
# speedup vs baseline: 1.0172x; 1.0172x over previous
"""Child-Sum TreeLSTM over complete binary trees — Trainium2 Bass kernel.

Sharding: data-parallel over the batch-of-trees axis B=32 across 8 NeuronCores
(4 trees/core); the 8 gate weight matrices are replicated.

Per-core dataflow (activations kept feature-transposed in SBUF as
[feat-chunk(128,128,44), 3, cols] tiles; weights natural = lhsT):
  - level-by-level bottom-up; per <=512-column block:
      embs^T loaded by XBAR DMA-transpose directly from a host-prepared
      bf16 padded copy of embs ([..., 384] with feature 300 = 1.0 ones row)
      for levels >= 7; PE-transpose path for the small deep levels
      gate pre-acts accumulate in PSUM over 6 K-chunks: x-side bf16 +
      h-side float32r; the combined bias (bx+bh) rides as a 45th weight row
      against the baked-in ones row
      sigma/tanh evacuate PSUM->SBUF in one ACT instruction per gate
      per-child forget gates use a step-0 duplicated rhs (each parent column
      streamed twice) so fx lands directly at child granularity
      c_new = i*u + f1*c1 + f2*c2 and h = o*tanh(c) on DVE
      h^T -> PE-transpose -> natural -> DMA to output
  - levels 10/9/8 spill h^T/c^T through internal DRAM (SBUF pressure);
    levels <= 7 keep h^T/c^T resident in SBUF
  - matmuls with moving dim < 256 (deep levels) switch the h-side to bf16
    (float32r drops to 4 cycles/row below 256).
"""

import numpy as np
import ml_dtypes

import concourse.bass as bass
import concourse.mybir as mybir
import concourse.tile as tile
from concourse import bacc
from concourse.masks import make_identity
from concourse.bass_utils import run_bass_kernel_spmd

F32 = mybir.dt.float32
F32R = mybir.dt.float32r
BF16 = mybir.dt.bfloat16
AF = mybir.ActivationFunctionType

B, D, DIM = 32, 11, 300
N = 2**D - 1          # 2047
CORES = 8
BL = B // CORES       # trees per core
KS = [128, 128, 44]   # feature chunks of 300
KO = [0, 128, 256]
NBMAX = 512
SPILL_LV = (10, 9, 8)
SPOFF = {10: 0, 9: BL * 1024, 8: BL * 1024 + BL * 512}
SPTOT = BL * 1024 + BL * 512 + BL * 256
PROJ = {"i": 0, "f": 1, "o": 2, "u": 3}

_NC_CACHE = []


def _cols(l):
    return BL * (1 << l)


def _build():
    nc = bacc.Bacc("TRN2", target_bir_lowering=False, debug=False,
                   num_devices=CORES)
    embs = nc.dram_tensor("embs", [BL, N, DIM], F32, kind="ExternalInput")
    WX = nc.dram_tensor("wx", [128, 4, 3, DIM], F32R, kind="ExternalInput")
    WH = nc.dram_tensor("wh", [128, 4, 3, DIM], F32R, kind="ExternalInput")
    hout = nc.dram_tensor("hout", [BL, N, DIM], F32, kind="ExternalOutput")
    sph = nc.dram_tensor("sph", [128, 3, SPTOT], F32R)
    spc = nc.dram_tensor("spc", [128, 3, SPTOT], F32)

    with tile.TileContext(nc) as tc:
        import contextlib
        with contextlib.ExitStack() as ctx:
            sb = ctx.enter_context(tc.tile_pool(name="sb", bufs=1))
            exp = ctx.enter_context(tc.tile_pool(name="exp", bufs=2))
            xtp = ctx.enter_context(tc.tile_pool(name="xtp", bufs=2))
            hsp = ctx.enter_context(tc.tile_pool(name="hsp", bufs=2))
            gp = ctx.enter_context(tc.tile_pool(name="gp", bufs=5))
            fcp = ctx.enter_context(tc.tile_pool(name="fcp", bufs=2))
            onp = ctx.enter_context(tc.tile_pool(name="onp", bufs=2))
            hcb = ctx.enter_context(tc.tile_pool(name="hcb", bufs=2))
            rbp = ctx.enter_context(tc.tile_pool(name="rbp", bufs=2))
            stp = ctx.enter_context(tc.tile_pool(name="stp", bufs=1))
            psum = ctx.enter_context(
                tc.tile_pool(name="psum", bufs=2, space="PSUM"))

            ident = sb.tile([128, 128], F32)
            make_identity(nc, ident[:, :])

            wx_t = sb.tile([128, 4, 3, DIM], F32R, name="wx_t")
            wh_t = sb.tile([128, 4, 3, DIM], F32R, name="wh_t")
            nc.sync.dma_start(out=wx_t[:, :, :, :], in_=WX[:, :, :, :])
            nc.sync.dma_start(out=wh_t[:, :, :, :], in_=WH[:, :, :, :])
            wx = {nm: wx_t[:, p] for nm, p in PROJ.items()}
            wh = {nm: wh_t[:, p] for nm, p in PROJ.items()}

            def nat_ap(dram, l, r0, rs, w):
                base = (1 << l) - 1
                if l >= 7:
                    t, j0 = r0 >> l, r0 & ((1 << l) - 1)
                    return dram[t, base + j0: base + j0 + rs, 0:w]
                t0, tcnt = r0 >> l, rs >> l
                return dram[t0:t0 + tcnt, base:base + (1 << l), 0:w]

            def load_ex(l, c0, nb):
                """embs^T (+ones row) for parent cols [c0, c0+nb), bf16."""
                ex = exp.tile([128, 3, NBMAX], F32R, tag="ex")
                pT = psum.tile([128, 3, NBMAX], F32, tag="big")
                for r0 in range(0, nb, 128):
                    rs = min(128, nb - r0)
                    xt = xtp.tile([128, 304], F32, tag="xt")
                    nc.gpsimd.memset(xt[:, 300:304], 1.0)
                    nc.sync.dma_start(out=xt[0:rs, 0:300],
                                      in_=nat_ap(embs, l, c0 + r0, rs, DIM))
                    for f in range(3):
                        ke = KS[f] + (1 if f == 2 else 0)
                        nc.tensor.transpose(
                            out=pT[0:ke, f, r0:r0 + rs],
                            in_=xt[0:rs, KO[f]:KO[f] + ke],
                            identity=ident[0:rs, 0:rs])
                nc.scalar.copy(ex[0:128, 0, 0:nb], pT[0:128, 0, 0:nb])
                nc.scalar.copy(ex[0:128, 1, 0:nb], pT[0:128, 1, 0:nb])
                nc.scalar.copy(ex[0:45, 2, 0:nb], pT[0:45, 2, 0:nb])
                return ex

            def store_nat(l, c0, nb, hsrc, s0):
                for r0 in range(0, nb, 128):
                    rs = min(128, nb - r0)
                    pO = psum.tile([128, 304], F32, tag="oT")
                    for f in range(3):
                        nc.tensor.transpose(
                            out=pO[0:rs, KO[f]:KO[f] + KS[f]],
                            in_=hsrc[0:KS[f], f,
                                     s0 + r0:s0 + r0 + rs].bitcast(F32),
                            identity=ident[0:KS[f], 0:KS[f]])
                    on = onp.tile([128, 300], F32, tag="on")
                    nc.scalar.copy(on[0:rs, :], pO[0:rs, 0:300])
                    nc.gpsimd.dma_start(out=nat_ap(hout, l, c0 + r0, rs,
                                                   DIM),
                                        in_=on[0:rs, :])

            st_h = {l: stp.tile([128, 3, _cols(l)], F32R, tag=f"sh{l}",
                                name=f"sh{l}") for l in range(0, 8)}
            st_c = {l: stp.tile([128, 3, _cols(l)], F32, tag=f"sc{l}",
                                name=f"sc{l}") for l in range(0, 8)}

            # ---------------- leaves (level 10) ----------------
            def leaf_block(c0):
                l, nb = 10, NBMAX
                ex = load_ex(l, c0, nb)
                sg = {}
                for nm, fn in (("i", AF.Sigmoid), ("o", AF.Sigmoid),
                               ("u", AF.Tanh)):
                    pG = psum.tile([128, 3, NBMAX], F32, tag="big",
                                   name=f"lpg_{c0}_{nm}")
                    for m in range(3):
                        ms, mo = KS[m], KO[m]
                        for k in range(3):
                            kx = KS[k] + (1 if k == 2 else 0)
                            nc.tensor.matmul(
                                pG[0:ms, m, 0:nb],
                                wx[nm][0:kx, k, mo:mo + ms],
                                ex[0:kx, k, 0:nb],
                                start=(k == 0), stop=(k == 2))
                    g = gp.tile([128, 3, NBMAX], F32, tag="g",
                                name=f"lg_{c0}_{nm}")
                    nc.scalar.activation(g[:, :, 0:nb], pG[:, :, 0:nb], fn)
                    sg[nm] = g
                cb = hcb.tile([128, 3, NBMAX], F32, tag="lc", bufs=3,
                              name=f"lc_{c0}")
                hb = hcb.tile([128, 3, NBMAX], F32R, tag="lh", bufs=3,
                              name=f"lh_{c0}")
                nc.vector.tensor_mul(cb[:, :, 0:nb], sg["i"][:, :, 0:nb],
                                     sg["u"][:, :, 0:nb])
                th = gp.tile([128, 3, NBMAX], F32, tag="g",
                             name=f"lth_{c0}")
                nc.scalar.activation(th[:, :, 0:nb], cb[:, :, 0:nb], AF.Tanh)
                nc.vector.tensor_mul(hb[:, :, 0:nb], sg["o"][:, :, 0:nb],
                                     th[:, :, 0:nb])
                store_nat(l, c0, nb, hb, 0)
                return hb, cb

            # ---------------- internal levels 9..0 ----------------
            for l in range(9, -1, -1):
                cols = _cols(l)
                spill = l in SPILL_LV
                child_spill = (l + 1) in SPILL_LV
                for c0 in range(0, cols, NBMAX):
                    nb = min(NBMAX, cols - c0)
                    fs = min(2 * nb, NBMAX)
                    nsub = (2 * nb) // fs
                    if l == 9:
                        leaf_hc = [leaf_block(2 * c0 + s * fs)
                                   for s in range(nsub)]
                    ex = load_ex(l, c0, nb)

                    hn, cn = [], []
                    for s in range(nsub):
                        ch0 = 2 * c0 + s * fs
                        if l == 9:
                            hn.append((leaf_hc[s][0], 0))
                            cn.append((leaf_hc[s][1], 0))
                        elif child_spill:
                            t_h = rbp.tile([128, 3, NBMAX], F32R, tag="rh")
                            t_c = rbp.tile([128, 3, NBMAX], F32, tag="rc")
                            off = SPOFF[l + 1] + ch0
                            nc.sync.dma_start(out=t_h[:, :, 0:fs],
                                              in_=sph[:, :, off:off + fs])
                            nc.sync.dma_start(out=t_c[:, :, 0:fs],
                                              in_=spc[:, :, off:off + fs])
                            hn.append((t_h, 0))
                            cn.append((t_c, 0))
                        else:
                            hn.append((st_h[l + 1], ch0))
                            cn.append((st_c[l + 1], ch0))

                    hs = hsp.tile([128, 3, NBMAX], F32R, tag="hs",
                                  name=f"hs_{l}_{c0}")
                    for s in range(nsub):
                        t_h, o_h = hn[s]
                        pair = t_h[:, :, o_h:o_h + fs].rearrange(
                            "p c (n two) -> p c n two", two=2)
                        nc.vector.tensor_add(
                            hs[:, :, s * fs // 2:(s + 1) * fs // 2],
                            pair[:, :, :, 0], pair[:, :, :, 1])

                    sg = {}
                    for nm, fn in (("i", AF.Sigmoid), ("o", AF.Sigmoid),
                                   ("u", AF.Tanh)):
                        pG = psum.tile([128, 3, NBMAX], F32, tag="big")
                        for m in range(3):
                            ms, mo = KS[m], KO[m]
                            for k in range(3):
                                kx = KS[k] + (1 if k == 2 else 0)
                                nc.tensor.matmul(
                                    pG[0:ms, m, 0:nb],
                                    wx[nm][0:kx, k, mo:mo + ms],
                                    ex[0:kx, k, 0:nb],
                                    start=(k == 0), stop=False)
                            for k in range(3):
                                nc.tensor.matmul(
                                    pG[0:ms, m, 0:nb],
                                    wh[nm][0:KS[k], k, mo:mo + ms],
                                    hs[0:KS[k], k, 0:nb],
                                    start=False, stop=(k == 2))
                        g = gp.tile([128, 3, NBMAX], F32, tag="g")
                        nc.scalar.activation(g[:, :, 0:nb], pG[:, :, 0:nb], fn)
                        sg[nm] = g

                    if spill:
                        cdst = hcb.tile([128, 3, NBMAX], F32, tag="cb")
                        hdst = hcb.tile([128, 3, NBMAX], F32R, tag="hb")
                        d0 = 0
                    else:
                        cdst, hdst, d0 = st_c[l], st_h[l], c0

                    cc = cdst[:, :, d0:d0 + nb]
                    nc.vector.tensor_mul(cc, sg["i"][:, :, 0:nb],
                                         sg["u"][:, :, 0:nb])

                    for s in range(nsub):
                        pF = psum.tile([128, 3, NBMAX], F32, tag="big")
                        p0 = s * fs // 2
                        w_h = wh["f"]
                        t_h, o_h = hn[s]
                        for m in range(3):
                            ms, mo = KS[m], KO[m]
                            for k in range(3):
                                kx = KS[k] + (1 if k == 2 else 0)
                                dup = ex[0:kx, k, p0:p0 + fs // 2] \
                                    .unsqueeze(2).broadcast_to([kx, fs // 2, 2])
                                nc.tensor.matmul(
                                    pF[0:ms, m, 0:fs],
                                    wx["f"][0:kx, k, mo:mo + ms], dup,
                                    start=(k == 0), stop=False)
                            for k in range(3):
                                nc.tensor.matmul(
                                    pF[0:ms, m, 0:fs],
                                    w_h[0:KS[k], k, mo:mo + ms],
                                    t_h[0:KS[k], k, o_h:o_h + fs],
                                    start=False, stop=(k == 2))
                        fg = gp.tile([128, 3, NBMAX], F32, tag="g")
                        nc.scalar.activation(fg[:, :, 0:fs], pF[:, :, 0:fs],
                                             AF.Sigmoid)
                        t_c, o_c = cn[s]
                        fc = fcp.tile([128, 3, NBMAX], F32, tag="fc")
                        nc.vector.tensor_mul(fc[:, :, 0:fs],
                                             fg[:, :, 0:fs],
                                             t_c[:, :, o_c:o_c + fs])
                        pair = fc[:, :, 0:fs].rearrange(
                            "p c (n two) -> p c n two", two=2)
                        ccs = cdst[:, :, d0 + p0:d0 + p0 + fs // 2]
                        nc.vector.tensor_add(ccs, ccs, pair[:, :, :, 0])
                        nc.vector.tensor_add(ccs, ccs, pair[:, :, :, 1])

                    th = gp.tile([128, 3, NBMAX], F32, tag="g")
                    nc.scalar.activation(th[:, :, 0:nb], cc, AF.Tanh)
                    nc.vector.tensor_mul(hdst[:, :, d0:d0 + nb],
                                         sg["o"][:, :, 0:nb], th[:, :, 0:nb])

                    if spill:
                        off = SPOFF[l] + c0
                        nc.gpsimd.dma_start(out=sph[:, :, off:off + nb],
                                            in_=hdst[:, :, 0:nb])
                        nc.gpsimd.dma_start(out=spc[:, :, off:off + nb],
                                            in_=cdst[:, :, 0:nb])
                    store_nat(l, c0, nb, hdst, d0)
    nc.compile()
    return nc


def kernel(embs, Wix, bix, Wih, bih, Wfx, bfx, Wfh, bfh,
           Wox, box, Woh, boh, Wux, bux, Wuh, buh):
    embs = np.ascontiguousarray(np.asarray(embs, dtype=np.float32))
    if not _NC_CACHE:
        _NC_CACHE.append(_build())
    nc = _NC_CACHE[0]

    def chunked(stack, bias_rows):
        out = np.zeros((128, 4, 3, DIM), np.float32)
        for p in range(4):
            out[0:128, p, 0] = stack[p][0:128]
            out[0:128, p, 1] = stack[p][128:256]
            out[0:44, p, 2] = stack[p][256:300]
            if bias_rows is not None:
                out[44, p, 2] = bias_rows[p]
        return out

    xw = [np.asarray(w, np.float32) for w in (Wix, Wfx, Wox, Wux)]
    xb = [np.asarray(bix) + np.asarray(bih), np.asarray(bfx) + np.asarray(bfh),
          np.asarray(box) + np.asarray(boh), np.asarray(bux) + np.asarray(buh)]
    hw_ = [np.asarray(w, np.float32) for w in (Wih, Wfh, Woh, Wuh)]
    wxp = chunked(xw, xb)
    whp = chunked(hw_, None)

    in_maps = [{"embs": embs[c * BL:(c + 1) * BL],
                "wx": wxp, "wh": whp}
               for c in range(CORES)]
    res = run_bass_kernel_spmd(nc, in_maps, list(range(CORES)))
    return np.concatenate([res.results[c]["hout"] for c in range(CORES)],
                          axis=0)
